# revision 16
# baseline (speedup 1.0000x reference)
"""Trainium2 Bass kernel for nn_MaxMinAgg.

Computes, for full inputs m [1024, 256] f32 and weight [256, 512] f32:
    z[b, j]  = max_k min(m[b, k], weight[k, j])          (tropical max-min matmul)
    out[b,o] = max_a z[b, 4*o + a]                       (max-pool over AGG=4 groups)

Key identity: max_a min(x, w_a) = min(x, max_a w_a): the AGG max-pool folds into
the weight (wmax[k, o] = max_a weight[k, 4o+a]), 4x less elementwise work, and
    out[b, o] = max_k min(m[b, k], wmax[k, o])
All ops are exact f32 selections -> bit-exact result.

Distribution: data-parallel over batch across 8 NeuronCores (128 rows each);
weight replicated.

Per-core algorithm. The elementwise min+max-reduce streams ~2 passes over
b*o*k/core on the DVE (the only engine with a 2-tensor min) - that is the time
floor; everything else hides under/around it:
  - Partitions carry p = kg*64 + og (kg in {0,1} k-halves, og in [0,64) output
    groups): partition p handles outputs o = t*64+og (2 o-blocks) and k-half
    [kg*128, kg*128+128).  m is DMA-broadcast from DRAM with only 64x
    replication (8MB) in 512B-contiguous runs, b-chunked so compute starts
    while m still streams.
  - Weight: one segmented reduce folds AGG -> wmax; two PE transposes ->
    wmaxT [o, k]; wmaxT round-trips through DRAM so per-o-block weight tiles
    wblock[p, k'] land in the partition layout (transpose outputs must start
    at PSUM partition 0, so direct placement is impossible).
  - Per o-block t: DVE tensor_tensor min (wblock free-broadcast over b vs
    mrep) + segmented tensor_reduce max over the k-half -> partial[p, b];
    PE-transpose partial and a tiny strided DVE max-reduce over the 2 kg
    slots emits out[b, t-block] in natural layout (no final transpose).
"""

import sys

import numpy as np

if "/opt/trn_rl_repo" not in sys.path:
    sys.path.insert(0, "/opt/trn_rl_repo")

B, IN_F, OUT_F, AGG = 1024, 256, 128, 4
N_CORES = 8
B_SH = B // N_CORES  # 128

KG, OG = 2, 64  # partition factorization: p = kg*OG + og
KS = IN_F // KG  # 128 k per group
NT = OUT_F // OG  # 2 o-blocks

# b-chunks (compute starts while m still streams in).
B_CHUNKS = [16, 32, 80]

_CACHE = {}


def emit_core_program(tc, o_d, m_d, w_d):
    """Emit the per-core Tile program.

    o_d: DRAM out [B_SH, OUT_F] f32, m_d: DRAM in [B_SH, IN_F] f32,
    w_d: DRAM in [IN_F, OUT_F*AGG] f32.
    """
    from contextlib import ExitStack

    import concourse.bass as bass
    from concourse import mybir
    from concourse.masks import make_identity

    nc = tc.nc
    f32 = mybir.dt.float32
    AX = mybir.AxisListType
    OP = mybir.AluOpType

    with ExitStack() as ctx:
        const = ctx.enter_context(tc.tile_pool(name="const", bufs=1))
        mintp = ctx.enter_context(tc.tile_pool(name="mintp", bufs=2))
        partp = ctx.enter_context(tc.tile_pool(name="partp", bufs=2))
        ps_tr = ctx.enter_context(tc.tile_pool(name="ps_tr", bufs=2, space="PSUM"))

        # --- weight load first (half per queue) ----------------------------
        w_sb = const.tile([128, 2, OUT_F * AGG], f32)
        wv = w_d.rearrange("(h p) j -> p h j", p=128)
        nc.scalar.dma_start(out=w_sb[:, 0, :], in_=wv[:, 0, :])
        nc.sync.dma_start(out=w_sb[:, 1, :], in_=wv[:, 1, :])

        # --- m broadcast: partition p = kg*OG+og gets m[b, kg*KS:(kg+1)*KS],
        # replicated over the 64 og's (8MB total, 512B contiguous runs).
        # One tile per b-chunk so compute unblocks per chunk.  All bulk rides
        # the scalar queue (the sync queue measures ~3x slower); the tiny
        # weight-side transfers ride sync so they never sit behind the bulk.
        mreps = []
        b0 = 0
        for ci, bc in enumerate(B_CHUNKS):
            mrep = const.tile([128, bc, KS], f32, name=f"mrep{ci}")
            for kg in range(KG):
                src = bass.AP(
                    tensor=m_d.tensor,
                    offset=m_d.offset + b0 * IN_F + kg * KS,
                    ap=[[0, OG], [IN_F, bc], [1, KS]],
                )
                nc.scalar.dma_start(
                    out=mrep[kg * OG : (kg + 1) * OG, :, :], in_=src
                )
            mreps.append(mrep)
            b0 += bc

        # --- weight fold: wmax[k_p, h, o] = max_a w[k, 4o+a] ---------------
        wmax_sb = const.tile([128, 2, OUT_F], f32)
        nc.vector.tensor_reduce(
            out=wmax_sb,
            in_=w_sb.rearrange("p h (o a) -> p h o a", a=AGG),
            axis=AX.X,
            op=OP.max,
        )

        ident = const.tile([128, 128], f32)
        make_identity(nc, ident)

        # wmaxT [o, k] via two PE transposes, then to DRAM so the per-block
        # weight tiles can be fetched in the p = kg*OG+og partition layout
        # (transpose outputs must land at PSUM partition 0, so direct
        # placement at partition offsets is impossible).
        wmaxT = const.tile([128, 2, 128], f32)
        for h in range(2):
            pt = ps_tr.tile([128, 128], f32, tag="ptr")
            nc.tensor.transpose(pt, wmax_sb[:, h, :], ident)
            nc.scalar.copy(out=wmaxT[:, h, :], in_=pt)
        wT_d = nc.dram_tensor("wT_scratch", [OUT_F, IN_F], f32, kind="Internal").ap()
        nc.sync.dma_start(out=wT_d, in_=wmaxT)

        # wblock_t[p=kg*OG+og, k'] = wmaxT[t*OG+og, kg*KS+k']
        wbs = []
        for t in range(NT):
            wb = const.tile([128, KS], f32, tag="wb", bufs=2, name=f"wb{t}")
            src = bass.AP(
                tensor=wT_d.tensor,
                offset=wT_d.offset + t * OG * IN_F,
                ap=[[KS, KG], [IN_F, OG], [1, KS]],
            )
            nc.sync.dma_start(out=wb, in_=src)
            wbs.append(wb)

        out_sb = const.tile([B_SH, OUT_F], f32)
        partials = [
            const.tile([128, B_SH], f32, name=f"partial{t}") for t in range(NT)
        ]

        # chunk-major: each m chunk is consumed for both o-blocks as soon as
        # it lands; DVE stays dense while later chunks stream in.
        b0 = 0
        for ci, bc in enumerate(B_CHUNKS):
            for t in range(NT):
                mint = mintp.tile([128, max(B_CHUNKS), KS], f32, tag="mint")
                nc.vector.tensor_tensor(
                    out=mint[:, :bc, :],
                    in0=wbs[t]
                    .rearrange("p k -> p () k")
                    .broadcast_to((128, bc, KS)),
                    in1=mreps[ci],
                    op=OP.min,
                )
                nc.vector.tensor_reduce(
                    out=partials[t][:, b0 : b0 + bc],
                    in_=mint[:, :bc, :],
                    axis=AX.X,
                    op=OP.max,
                )
            b0 += bc

        # transpose partial [p, b] -> [b, p], combine the KG kg-slots
        for t in range(NT):
            ptr = ps_tr.tile([128, 128], f32, tag="ptr")
            nc.tensor.transpose(ptr, partials[t], ident)
            nc.vector.tensor_reduce(
                out=out_sb[:, t * OG : (t + 1) * OG],
                in_=ptr.rearrange("b (kg og) -> b og kg", kg=KG),
                axis=AX.X,
                op=OP.max,
            )

        nc.sync.dma_start(out=o_d, in_=out_sb)


def _build():
    if "nc" in _CACHE:
        return _CACHE["nc"]
    import concourse.bacc as bacc
    import concourse.tile as tile
    from concourse import mybir

    f32 = mybir.dt.float32
    nc = bacc.Bacc(
        "TRN2",
        target_bir_lowering=False,
        debug=False,
        enable_asserts=True,
        num_devices=N_CORES,
    )
    m_d = nc.dram_tensor("m0", [B_SH, IN_F], f32, kind="ExternalInput").ap()
    w_d = nc.dram_tensor("w0", [IN_F, OUT_F * AGG], f32, kind="ExternalInput").ap()
    o_d = nc.dram_tensor("out0", [B_SH, OUT_F], f32, kind="ExternalOutput").ap()
    with tile.TileContext(nc) as tc:
        emit_core_program(tc, o_d, m_d, w_d)
    nc.compile()
    _CACHE["nc"] = nc
    return nc


def run(m, weight, trace=False, **spmd_kwargs):
    """Run on 8 NeuronCores; returns (full_output, BassKernelResults)."""
    from concourse.bass_utils import run_bass_kernel_spmd

    nc = _build()
    m = np.ascontiguousarray(np.asarray(m, dtype=np.float32))
    weight = np.ascontiguousarray(np.asarray(weight, dtype=np.float32))
    assert m.shape == (B, IN_F) and weight.shape == (IN_F, OUT_F * AGG)
    in_maps = [
        {"m0": m[i * B_SH : (i + 1) * B_SH], "w0": weight} for i in range(N_CORES)
    ]
    res = run_bass_kernel_spmd(
        nc, in_maps, core_ids=list(range(N_CORES)), trace=trace, **spmd_kwargs
    )
    out = np.concatenate([res.results[i]["out0"] for i in range(N_CORES)], axis=0)
    return out, res


def kernel(m, weight, agg_features=AGG, **_ignored):
    assert int(agg_features) == AGG
    out, _ = run(m, weight, trace=False)
    return out.astype(np.float32)
